# revision 1
# baseline (speedup 1.0000x reference)
"""GAT (graph attention) kernel for Trainium2, 8-core SPMD.

Strategy:
  - Nodes sharded 8 ways (12500/core, padded to 12544 = 98*128).
  - Stage A: h = X @ W^T on PE (per-tile PE transpose + matmul against
    host-pretransposed W); e_src/e_dst head dots on DVE. Each core writes a
    [12544, 256]-bf16 table slab: row = [h bf16 x128 | e_dst f32 bits x4 |
    pad]. Pad-node rows get e_dst = -1e30 (softmax-transparent dummies).
  - Stage B: AllGather slabs -> full 100352-row table per core.
  - Stage C: per 128-node tile, the 16 neighbor rows per node are fetched
    with dma_gather (custom gather instr, int16 indices). The table is
    covered by 4 windows of <=32768 rows; each node's neighbors are sorted
    so window-w neighbors occupy a contiguous slot range; per-tile slot
    counts J[t,w] are equalized by host-side node reordering (nodes sorted
    by window-count profile) and padded with dummy rows. Softmax over the
    slots (dummies vanish via e=-1e30), weighted sum over slots, ELU.
  - The per-tile slot table J is data-dependent: the bass program is built
    and compiled on first kernel() call (cached by J-table hash).

kernel(**inputs) does host-side sharding/index prep only (no FP math on
tensor data), runs the SPMD program, and reassembles the full output.
"""
import sys

if "/opt/trn_rl_repo" not in sys.path:
    sys.path.insert(0, "/opt/trn_rl_repo")

import hashlib
import numpy as np

N, DEG, K, F_IN, F_OUT = 100000, 16, 4, 128, 32
KF = K * F_OUT            # 128
N_CORES = 8
S = N // N_CORES          # 12500
P = 128
NT = (S + P - 1) // P     # 98
SP = NT * P               # 12544
NTAB = N_CORES * SP       # 100352
EL = 256                  # bf16 elements per table row (512B)
EOFF = 128                # h at [0,128); e_dst hi bf16 [128,132); lo [132,136)
BOUNDS = (0, 2 * SP, 4 * SP, 6 * SP, NTAB)   # shard-pair aligned (25088)
NW = 4
# dummy rows: first pad row of shards 0,2,4,6 (one inside each window)
DUMMY = (S, 2 * SP + S, 4 * SP + S, 6 * SP + S)
NEG_SLOPE = 0.01
NEG_BIG = -1.0e30


def build_nc(J, n_cores=N_CORES, nt=NT, distributed=True):
    """J: [nt, NW] int array of per-tile window slot counts (uniform across
    cores). Builds and compiles the SPMD program."""
    from contextlib import ExitStack

    import concourse.bass as bass
    import concourse.tile as tile
    from concourse import bacc, mybir
    from concourse.masks import make_identity

    f32 = mybir.dt.float32
    bf16 = mybir.dt.bfloat16
    i16 = mybir.dt.int16
    sp = nt * P
    Jt = J.sum(axis=1)            # slots per tile
    CTOT = int(J.sum()) * 8       # idxbuf columns (16-wrapped)

    nc = bacc.Bacc("TRN2", target_bir_lowering=False, debug=False,
                   num_devices=n_cores, num_swdge_queues=4)

    xs = nc.dram_tensor("xs", [sp, F_IN], f32, kind="ExternalInput")
    wt = nc.dram_tensor("wt", [F_IN, KF], f32, kind="ExternalInput")
    avec = nc.dram_tensor("avec", [2, KF], f32, kind="ExternalInput")
    idxin = nc.dram_tensor("idxin", [P, CTOT], i16, kind="ExternalInput")
    padfill = nc.dram_tensor("padfill", [sp - S if sp > S else 1, 8], bf16,
                             kind="ExternalInput")
    out = nc.dram_tensor("out", [sp, KF], f32, kind="ExternalOutput")

    he_shard = nc.dram_tensor("he_shard", [sp, EL], bf16, kind="Internal")
    he_full = nc.dram_tensor("he_full", [NTAB, EL], bf16, kind="Internal",
                             addr_space="Shared" if distributed else "Local")

    with tile.TileContext(nc) as tc, ExitStack() as ctx:
        consts = ctx.enter_context(tc.tile_pool(name="consts", bufs=1))
        sa = ctx.enter_context(tc.tile_pool(name="sa", bufs=3))
        sa_ps = ctx.enter_context(tc.tile_pool(name="sa_ps", bufs=2, space="PSUM"))
        sc = ctx.enter_context(tc.tile_pool(name="sc", bufs=3))
        scg = ctx.enter_context(tc.tile_pool(name="scg", bufs=3))

        ident = consts.tile([P, P], f32)
        make_identity(nc, ident[:])
        wt_sb = consts.tile([F_IN, KF], f32)
        nc.sync.dma_start(wt_sb[:], wt.ap())
        av_sb = consts.tile([P, 2 * KF], f32)
        nc.sync.dma_start(av_sb[:], bass.AP(avec, 0, [[0, P], [1, 2 * KF]]))
        asrc_b = av_sb[:, 0:KF]
        adst_b = av_sb[:, KF:2 * KF]
        es_sb = consts.tile([P, nt * K], f32)
        idx_sb = consts.tile([P, CTOT], i16)
        nc.sync.dma_start(idx_sb[:], idxin.ap())

        # ---- Stage A ----
        for t in range(nt):
            x_t = sa.tile([P, F_IN], f32, tag="x")
            nc.sync.dma_start(x_t[:], xs.ap()[t * P:(t + 1) * P, :])
            xt_ps = sa_ps.tile([P, P], f32, tag="xt")
            nc.tensor.transpose(out=xt_ps[:], in_=x_t[:], identity=ident[:])
            xt_sb = sa.tile([P, P], f32, tag="xt_sb")
            nc.vector.tensor_copy(xt_sb[:], xt_ps[:])
            h_ps = sa_ps.tile([P, KF], f32, tag="h")
            nc.tensor.matmul(h_ps[:], lhsT=xt_sb[:], rhs=wt_sb[:],
                             start=True, stop=True)
            he_t = sa.tile([P, EL], bf16, tag="he")
            nc.scalar.copy(he_t[:, 0:KF], h_ps[:])          # f32 -> bf16
            nc.vector.memset(he_t[:, EOFF + 8:EL], 0.0)
            tmp = sa.tile([P, KF], f32, tag="tmp")
            nc.vector.tensor_mul(tmp[:], h_ps[:], adst_b)
            e32 = sa.tile([P, K], f32, tag="e32")
            nc.vector.reduce_sum(
                e32[:], tmp[:].rearrange("p (k f) -> p k f", f=F_OUT),
                axis=mybir.AxisListType.X)
            # e_dst stored as hi+lo bf16 pair (~16-bit mantissa total)
            nc.vector.tensor_copy(he_t[:, EOFF:EOFF + 4], e32[:])
            ehi32 = sa.tile([P, K], f32, tag="ehi32")
            nc.vector.tensor_copy(ehi32[:], he_t[:, EOFF:EOFF + 4])
            elo = sa.tile([P, K], f32, tag="elo")
            nc.vector.tensor_sub(elo[:], e32[:], ehi32[:])
            nc.vector.tensor_copy(he_t[:, EOFF + 4:EOFF + 8], elo[:])
            tmp2 = sa.tile([P, KF], f32, tag="tmp2")
            nc.vector.tensor_mul(tmp2[:], h_ps[:], asrc_b)
            nc.vector.reduce_sum(
                es_sb[:, t * K:(t + 1) * K],
                tmp2[:].rearrange("p (k f) -> p k f", f=F_OUT),
                axis=mybir.AxisListType.X)
            nc.sync.dma_start(he_shard.ap()[t * P:(t + 1) * P, :], he_t[:])
        # pad rows are window dummies: e_dst <- -1e30 (after slab writes)
        npad = sp - S
        if npad > 0:
            pf = consts.tile([npad, 8], bf16)
            nc.sync.dma_start(pf[:], padfill.ap())
            nc.sync.dma_start(
                he_shard.ap()[S:sp, EOFF:EOFF + 8], pf[:])

        # ---- Stage B ----
        if distributed:
            nc.gpsimd.collective_compute(
                "AllGather", mybir.AluOpType.bypass,
                replica_groups=[list(range(n_cores))],
                ins=[he_shard.ap()], outs=[he_full.ap()])
        else:
            for t in range(nt):
                cp = sa.tile([P, EL], bf16, tag="cp")
                nc.sync.dma_start(cp[:], he_shard.ap()[t * P:(t + 1) * P, :])
                nc.sync.dma_start(he_full.ap()[t * P:(t + 1) * P, :], cp[:])

        # ---- Stage C ----
        coff = 0
        ncall = 0
        for t in range(nt):
            jt = int(Jt[t])
            g = scg.tile([P, jt * EL], bf16, tag="g")
            g3 = g[:].rearrange("p (d w) -> p d w", w=EL)
            b = 0
            for w in range(NW):
                jw = int(J[t, w])
                if jw == 0:
                    continue
                nidx = jw * P
                nc.gpsimd.dma_gather(
                    out_ap=g3[:, b:b + jw, :],
                    in_ap=he_full.ap()[BOUNDS[w]:BOUNDS[w + 1], :],
                    idxs_ap=idx_sb[:, coff:coff + jw * 8],
                    num_idxs=nidx, num_idxs_reg=nidx, elem_size=EL,
                    single_packet=False, queue_num=ncall % 4)
                ncall += 1
                b += jw
                coff += jw * 8
            # scores (k-major over slots): e_dst = hi + lo, then + e_src
            ehi = g3[:, :, EOFF:EOFF + 4].rearrange("p d k -> p k d")
            elo_v = g3[:, :, EOFF + 4:EOFF + 8].rearrange("p d k -> p k d")
            ed = sc.tile([P, K * jt], f32, tag="ed")
            edv = ed[:].rearrange("p (k d) -> p k d", d=jt)
            nc.vector.tensor_add(edv, ehi, elo_v)
            esrc = es_sb[:, t * K:(t + 1) * K].unsqueeze(-1).to_broadcast(
                [P, K, jt])
            s0 = sc.tile([P, K * jt], f32, tag="s0")
            s0v = s0[:].rearrange("p (k d) -> p k d", d=jt)
            nc.vector.tensor_add(s0v, edv, esrc)
            s1 = sc.tile([P, K * jt], f32, tag="s1")
            nc.vector.scalar_tensor_tensor(
                s1[:], s0[:], NEG_SLOPE, s0[:],
                op0=mybir.AluOpType.mult, op1=mybir.AluOpType.max)
            s1v = s1[:].rearrange("p (k d) -> p k d", d=jt)
            m = sc.tile([P, K], f32, tag="m")
            nc.vector.reduce_max(m[:], s1v, axis=mybir.AxisListType.X)
            s2 = sc.tile([P, K * jt], f32, tag="s2")
            nc.vector.tensor_sub(
                s2[:].rearrange("p (k d) -> p k d", d=jt), s1v,
                m[:].unsqueeze(-1).to_broadcast([P, K, jt]))
            pr = sc.tile([P, K * jt], f32, tag="pr")
            nc.scalar.activation(pr[:], s2[:], mybir.ActivationFunctionType.Exp)
            z = sc.tile([P, K], f32, tag="z")
            nc.vector.reduce_sum(z[:],
                                 pr[:].rearrange("p (k d) -> p k d", d=jt),
                                 axis=mybir.AxisListType.X)
            rz = sc.tile([P, K], f32, tag="rz")
            nc.vector.reciprocal(rz[:], z[:])
            al = sc.tile([P, K * jt], bf16, tag="al")
            nc.vector.tensor_mul(
                al[:].rearrange("p (k d) -> p k d", d=jt),
                pr[:].rearrange("p (k d) -> p k d", d=jt),
                rz[:].unsqueeze(-1).to_broadcast([P, K, jt]))
            # weighted h, written kf-major so the slot-reduce is unit-stride
            wg = scg.tile([P, KF * jt], bf16, tag="wg")
            nc.vector.tensor_mul(
                wg[:].rearrange("p (k f d) -> p d k f", k=K, f=F_OUT),
                g3[:, :, 0:KF].rearrange("p d (k f) -> p d k f", f=F_OUT),
                al[:].rearrange("p (k d) -> p d k", d=jt)
                    .unsqueeze(-1).to_broadcast([P, jt, K, F_OUT]))
            o = sc.tile([P, KF], f32, tag="o")
            nc.vector.reduce_sum(
                o[:], wg[:].rearrange("p (kf d) -> p kf d", d=jt),
                axis=mybir.AxisListType.X)
            # ELU(o) = max(o,0) + exp(min(o,0)) - 1
            t1 = sc.tile([P, KF], f32, tag="t1")
            nc.vector.tensor_scalar_min(t1[:], o[:], 0.0)
            e1 = sc.tile([P, KF], f32, tag="e1")
            nc.scalar.activation(e1[:], t1[:], mybir.ActivationFunctionType.Exp)
            r = sc.tile([P, KF], f32, tag="r")
            nc.vector.tensor_scalar_max(r[:], o[:], 0.0)
            ot = sc.tile([P, KF], f32, tag="ot")
            nc.vector.scalar_tensor_tensor(
                ot[:], e1[:], -1.0, r[:],
                op0=mybir.AluOpType.add, op1=mybir.AluOpType.add)
            nc.sync.dma_start(out.ap()[t * P:(t + 1) * P, :], ot[:])

    nc.compile()
    return nc


def host_plan(nbr):
    """Per-core node ordering, global J table, per-core idx buffers.
    Windows are shard-pair-aligned, so a neighbor's window depends only on
    its source core -- node reordering within shards cannot change it."""
    nbr = np.asarray(nbr).astype(np.int64)
    src_core = nbr // S
    # pass 1: window counts (window = source core pair) -> per-core order
    win = src_core // 2                                     # [N, DEG] in 0..3
    orders = []
    cnts = []
    for c in range(N_CORES):
        w = win[c * S:(c + 1) * S]
        cnt = np.stack([(w == q).sum(1) for q in range(NW)], 1)  # [S,NW]
        order = np.lexsort((cnt[:, 3], cnt[:, 2], cnt[:, 1], cnt[:, 0]))
        orders.append(order)
        cnts.append(cnt)
    # pass 2: table row of neighbor j = c_j*SP + inv_order_{c_j}(j % S)
    inv = np.empty(N, np.int64)
    for c in range(N_CORES):
        inv[c * S + orders[c]] = np.arange(S)
    rows = (src_core * SP + inv[nbr]).astype(np.int32)
    percore = []
    Js = np.zeros((N_CORES, NT, NW), np.int64)
    for c in range(N_CORES):
        r = np.sort(rows[c * S:(c + 1) * S], axis=1)[orders[c]]  # [S,16]
        cnt = cnts[c][orders[c]]
        rs = np.concatenate([r, np.zeros((SP - S, DEG), np.int32)])
        cs = np.concatenate([cnt, np.zeros((SP - S, NW), np.int64)])
        start = np.concatenate(
            [np.zeros((SP, 1), np.int64), np.cumsum(cs, 1)[:, :-1]], 1)
        percore.append((rs, cs, start))
        Js[c] = cs.reshape(NT, P, NW).max(1)
    J = Js.max(axis=0)                                      # [NT, NW]
    idxbufs = []
    for c in range(N_CORES):
        rs, cs, start = percore[c]
        segs = []
        for t in range(NT):
            rt = rs[t * P:(t + 1) * P]
            ct = cs[t * P:(t + 1) * P]
            st = start[t * P:(t + 1) * P]
            for w in range(NW):
                jw = int(J[t, w])
                if jw == 0:
                    continue
                jj = np.arange(jw)[None, :]
                take = st[:, w:w + 1] + jj
                valid = jj < ct[:, w:w + 1]
                vals = np.where(
                    valid,
                    np.take_along_axis(rt, np.minimum(take, DEG - 1).astype(
                        np.int64), 1),
                    DUMMY[w]).astype(np.int64) - BOUNDS[w]
                # linear order position i = j*128 + p -> 16-partition wrap
                lin = vals.T.reshape(-1)                     # [jw*128]
                seg = lin.reshape(-1, 16).T.astype(np.int16)  # [16, jw*8]
                segs.append(seg)
        buf16 = np.concatenate(segs, axis=1)
        idxbufs.append(np.ascontiguousarray(np.tile(buf16, (8, 1))))
    return J, orders, idxbufs


def prep_inputs(X, W, a, nbr):
    X = np.asarray(X, dtype=np.float32)
    W = np.asarray(W, dtype=np.float32)
    a = np.asarray(a, dtype=np.float32)
    J, orders, idxbufs = host_plan(nbr)
    wt = np.ascontiguousarray(W.transpose(2, 0, 1).reshape(F_IN, KF))
    avec = np.ascontiguousarray(
        np.stack([a[:, 0, :F_OUT].reshape(KF), a[:, 0, F_OUT:].reshape(KF)]))
    import ml_dtypes
    pf = np.full((max(SP - S, 1), 8), NEG_BIG, dtype=ml_dtypes.bfloat16)
    in_maps = []
    for c in range(N_CORES):
        xs = np.zeros((SP, F_IN), dtype=np.float32)
        xs[:S] = X[c * S:(c + 1) * S][orders[c]]
        in_maps.append({"xs": xs, "wt": wt, "avec": avec, "idxin": idxbufs[c],
                        "padfill": pf})
    return J, orders, in_maps


_NC_CACHE = {}


def kernel(X, W, a, nbr):
    from concourse.bass_utils import run_bass_kernel_spmd

    J, orders, in_maps = prep_inputs(X, W, a, nbr)
    key = hashlib.sha1(J.tobytes()).hexdigest()
    if key not in _NC_CACHE:
        _NC_CACHE[key] = build_nc(J)
    nc = _NC_CACHE[key]
    res = run_bass_kernel_spmd(nc, in_maps, core_ids=list(range(N_CORES)))
    out = np.empty((N, KF), dtype=np.float32)
    for c in range(N_CORES):
        out[c * S + orders[c]] = res.results[c]["out"][:S]
    return out

